# revision 6
# baseline (speedup 1.0000x reference)
"""Trainium2 Bass kernel for batched multi-head graph attention (GAT).

Reference computation (per batch b, head h):
    h_prime = h[b] @ w[h]                      # [N, FOUT]
    t = tanh(h_prime)
    src = t @ a_src[h]; dst = t @ a_dst[h]     # [N]
    s[i,j] = leaky_relu_{0.2}(src[i] + dst[j])
    attn = softmax_j(where(adj[b]>0, s, -inf))
    out[b,h] = attn @ h_prime

Device algorithm (core c <-> batch b=c):
    exp(leaky_relu(s)) = max(e^s, e^{0.2 s}); with s = src_i + dst_j the
    unnormalized weight factors as
        w[j,i] = adjT[j,i] * e^{src_i} * q_j * max(1, u_i * v_j)
    with u = e^{-0.8 src}, v = e^{-0.8 dst}, q = e^{dst}. e^{src_i} cancels in
    the softmax ratio and is never computed. Folding q into the DVE pass:
        mx_q[j,i] = max(u_i * vq_j, q_j)  (= q_j * max(1, u_i v_j)),
        vq = e^{0.2 dst}
    so the matmul stationary is just [h_prime | 1]; row 64 accumulates the
    softmax denominator. Per j-chunk only two DVE ops run (TS 4x + TT 2x).

    Projections src/dst are computed on the PE: head pairs are packed as a
    [128, 128] stationary (2 heads x 64 features) producing tanh(h_prime)^T
    in [f, n] layout, then a [128, 4] block-diagonal projection matmul gives
    src/dst rows for both heads at once.  dst rows are PE-transposed into
    per-chunk columns for the per-partition scalars of the TS op.

    The host divides rows 0..63 by row 64 and transposes to [b, h, n, f].
"""

import numpy as np
import ml_dtypes

import concourse.mybir as mybir
import concourse.tile as tile
from concourse import bacc
from concourse.bass_utils import run_bass_kernel_spmd

BS, N, FIN, NH, FOUT = 8, 1024, 256, 8, 64
P = 128
NCH = N // P          # 8 chunks of the node axis
KC = FIN // P         # 2 chunks of the feature-in axis
NPAIR = NH // 2       # head pairs for the packed projection matmuls
F32 = mybir.dt.float32
BF16 = mybir.dt.bfloat16
ALU = mybir.AluOpType
ACTF = mybir.ActivationFunctionType
BF16NP = ml_dtypes.bfloat16

HLF = 512  # moving-operand half width (PSUM bank limit at fp32 out)


def emit(nc, tc, hT_d, w_d, a4_d, adjT_d, ident_d, out_d):
    with (
        tc.tile_pool(name="const", bufs=1) as cpool,
        tc.tile_pool(name="t2", bufs=3) as tpool,
        tc.tile_pool(name="mx", bufs=6) as mxpool,
        tc.tile_pool(name="z", bufs=6) as zpool,
        tc.tile_pool(name="osb", bufs=2) as opool,
    ):
        # ---- constant tiles ----
        hT = cpool.tile([P, KC, N], BF16)
        wsb = cpool.tile([P, KC, NH * FOUT], BF16)
        a4 = cpool.tile([P, NPAIR, 4], BF16)
        adjT = cpool.tile([P, NCH, N], BF16)
        ident = cpool.tile([P, P], F32)
        ones_row = cpool.tile([1, P], BF16)
        nc.vector.memset(ones_row, 1.0)

        # hp stationary per j-chunk: [h, 66] (cols 0:64 h_prime, col 64 ones)
        hp_tiles = [cpool.tile([P, NH, 66], BF16, name=f"hp{ic}") for ic in range(NCH)]
        for ic in range(NCH):
            nc.vector.memset(hp_tiles[ic][:, :, 64:66], 1.0)

        u_tiles = [cpool.tile([1, N], BF16, name=f"u{h}") for h in range(NH)]
        dstp = [cpool.tile([2, N], F32, name=f"dp{p}") for p in range(NPAIR)]
        dst_rows = cpool.tile([NH, N], F32)
        q_col = cpool.tile([P, NCH, NH], F32)
        vq_col = cpool.tile([P, NCH, NH], F32)
        ub_tiles = [cpool.tile([P, N], BF16, name=f"ub{h}") for h in range(NH)]
        warm_src = cpool.tile([P, HLF], BF16)
        nc.vector.memset(warm_src, 0.0)

        # ---- input DMAs: few big transfers, split across both HWDGE queues ----
        # sync: what phase A needs first;  scalar: the big adj matrix.
        nc.sync.dma_start(wsb, w_d.rearrange("kc p f -> p kc f"))
        for kc in range(KC):
            nc.sync.dma_start(hT[:, kc, :], hT_d[kc])
        nc.sync.dma_start(a4, a4_d.rearrange("q p f -> p q f"))
        nc.sync.dma_start(ident, ident_d)
        nc.scalar.dma_start(adjT, adjT_d.rearrange("jc p n -> p jc n"))

        with (
            tc.tile_pool(name="psa1", bufs=2, space="PSUM") as pp_a1,
            tc.tile_pool(name="psa2", bufs=2, space="PSUM") as pp_a2,
            tc.tile_pool(name="psp", bufs=3, space="PSUM") as pp_p,
            tc.tile_pool(name="psdt", bufs=1, space="PSUM") as pp_dt,
        ):
            # ---- PE warm-up during the input-DMA window: trips the HAM
            # activity window to K=8/8 before the real matmuls start.
            warm = pp_a1.tile([P, HLF], F32, tag="a1", name="warm")
            for i in range(8):
                nc.tensor.matmul(warm, warm_src[:, 0:P], warm_src, start=True, stop=True)

            # ---- phase A2/A3: tanh(h_prime)^T per head pair + projections ----
            for pr in range(NPAIR):
                t2 = tpool.tile([P, N], BF16)
                for hf in range(2):
                    ps2 = pp_a2.tile([P, HLF], F32, tag="a2")
                    for kc in range(KC):
                        nc.tensor.matmul(
                            ps2,
                            wsb[:, kc, 2 * pr * FOUT : (2 * pr + 2) * FOUT],
                            hT[:, kc, hf * HLF : (hf + 1) * HLF],
                            start=(kc == 0),
                            stop=(kc == KC - 1),
                        )
                    nc.scalar.activation(
                        t2[:, hf * HLF : (hf + 1) * HLF], ps2, ACTF.Tanh
                    )
                # src/dst projections; every PSUM output lands at partition 0
                # (engines cannot shift partition bases; DMA reassembles dst).
                for hf in range(2):
                    sl = slice(hf * HLF, (hf + 1) * HLF)
                    for k in range(2):
                        psu = pp_p.tile([1, HLF], F32, tag="p", name=f"psu{pr}{hf}{k}")
                        nc.tensor.matmul(
                            psu, a4[:, pr, k : k + 1], t2[:, sl], start=True, stop=True
                        )
                        nc.scalar.activation(
                            u_tiles[2 * pr + k][:, sl], psu, ACTF.Exp, scale=-0.8
                        )
                    psd = pp_p.tile([2, HLF], F32, tag="p", name=f"psd{pr}{hf}")
                    nc.tensor.matmul(
                        psd, a4[:, pr, 2:4], t2[:, sl], start=True, stop=True
                    )
                    nc.scalar.activation(dstp[pr][:, sl], psd, ACTF.Copy)
                nc.sync.dma_start(dst_rows[2 * pr : 2 * pr + 2, :], dstp[pr])

            # ---- dst columns: PE-transpose dst rows into per-chunk scalars ----
            dstT = pp_dt.tile([P, NCH, NH], F32)
            for jc in range(NCH):
                nc.tensor.transpose(
                    dstT[:, jc, :],
                    dst_rows[:, jc * P : (jc + 1) * P],
                    ident[0:NH, 0:NH],
                )
            nc.scalar.activation(q_col, dstT, ACTF.Exp)
            nc.scalar.activation(vq_col, dstT, ACTF.Exp, scale=0.2)

            # ---- ub broadcast per head (PE ones outer product) ----
            for h in range(NH):
                for hf in range(2):
                    ubps = pp_p.tile([P, HLF], F32, tag="p", name=f"ub{h}{hf}")
                    nc.tensor.matmul(
                        ubps,
                        ones_row,
                        u_tiles[h][:, hf * HLF : (hf + 1) * HLF],
                        start=True,
                        stop=True,
                    )
                    nc.scalar.activation(
                        ub_tiles[h][:, hf * HLF : (hf + 1) * HLF], ubps, ACTF.Copy
                    )

            # ---- phase A1: h_prime in [n, h*f] layout for the stationaries ----
            for ic in range(NCH):
                ps1 = pp_a1.tile([P, NH * FOUT], F32, tag="a1")
                for kc in range(KC):
                    nc.tensor.matmul(
                        ps1,
                        hT[:, kc, ic * P : (ic + 1) * P],
                        wsb[:, kc, :],
                        start=(kc == 0),
                        stop=(kc == KC - 1),
                    )
                nc.scalar.activation(
                    hp_tiles[ic][:, :, 0:FOUT],
                    ps1.rearrange("p (h f) -> p h f", f=FOUT),
                    ACTF.Copy,
                )

        # ---- phase C: masked weights + attention matmuls ----
        with tc.tile_pool(name="psout", bufs=4, space="PSUM") as pp_out:
            for h in range(NH):
                pso = [
                    pp_out.tile([FOUT + 1, HLF], F32, tag="out", name=f"o{h}_{hf}")
                    for hf in range(2)
                ]
                for jc2 in range(NCH // 2):
                    mx = mxpool.tile([P, 2, N], BF16)
                    for k in range(2):
                        jc = 2 * jc2 + k
                        nc.vector.tensor_scalar(
                            mx[:, k, :],
                            ub_tiles[h],
                            vq_col[:, jc, h : h + 1],
                            q_col[:, jc, h : h + 1],
                            ALU.mult,
                            ALU.max,
                        )
                    z = zpool.tile([P, 2, N], BF16)
                    nc.vector.tensor_tensor(
                        z, mx, adjT[:, 2 * jc2 : 2 * jc2 + 2, :], ALU.mult
                    )
                    for k in range(2):
                        jc = 2 * jc2 + k
                        for hf in range(2):
                            nc.tensor.matmul(
                                pso[hf],
                                hp_tiles[jc][:, h, 0:65],
                                z[:, k, hf * HLF : (hf + 1) * HLF],
                                start=(jc == 0),
                                stop=(jc == NCH - 1),
                            )
                ot = opool.tile([FOUT + 1, N], F32)
                nc.scalar.activation(ot[:, 0:HLF], pso[0], ACTF.Copy)
                nc.scalar.activation(ot[:, HLF:N], pso[1], ACTF.Copy)
                nc.sync.dma_start(out_d[h], ot)


def build_program(num_devices=8, debug=False):
    nc = bacc.Bacc(
        "TRN2", target_bir_lowering=False, debug=debug, num_devices=num_devices
    )
    hT_d = nc.dram_tensor("hT", [KC, P, N], BF16, kind="ExternalInput").ap()
    w_d = nc.dram_tensor("w_all", [KC, P, NH * FOUT], BF16, kind="ExternalInput").ap()
    a4_d = nc.dram_tensor("a4", [NPAIR, P, 4], BF16, kind="ExternalInput").ap()
    adjT_d = nc.dram_tensor("adjT", [NCH, P, N], BF16, kind="ExternalInput").ap()
    ident_d = nc.dram_tensor("ident", [P, P], F32, kind="ExternalInput").ap()
    out_d = nc.dram_tensor("outT", [NH, FOUT + 1, N], F32, kind="ExternalOutput").ap()
    with tile.TileContext(nc) as tc:
        emit(nc, tc, hT_d, w_d, a4_d, adjT_d, ident_d, out_d)
    nc.compile()
    return nc


def make_in_maps(h, adj, w, a_src, a_dst):
    """Host-side sharding/layout prep: core c gets batch c."""
    w_all = np.ascontiguousarray(
        w.astype(np.float32).transpose(1, 0, 2).reshape(KC, P, NH * FOUT)
    ).astype(BF16NP)
    a4 = np.zeros((NPAIR, P, 4), dtype=np.float32)
    for pr in range(NPAIR):
        a4[pr, 0:FOUT, 0] = a_src[2 * pr, :, 0]
        a4[pr, FOUT:P, 1] = a_src[2 * pr + 1, :, 0]
        a4[pr, 0:FOUT, 2] = a_dst[2 * pr, :, 0]
        a4[pr, FOUT:P, 3] = a_dst[2 * pr + 1, :, 0]
    a4 = a4.astype(BF16NP)
    ident = np.eye(P, dtype=np.float32)
    in_maps = []
    for b in range(BS):
        hT = np.ascontiguousarray(
            h[b].astype(np.float32).T.reshape(KC, P, N)
        ).astype(BF16NP)
        adjT = np.ascontiguousarray(adj[b].T.reshape(NCH, P, N)).astype(BF16NP)
        in_maps.append(
            {"hT": hT, "w_all": w_all, "a4": a4, "adjT": adjT, "ident": ident}
        )
    return in_maps


def postprocess(raw_outs):
    """raw_outs: list of [NH, FOUT+1, N] per core -> full [BS, NH, N, FOUT]."""
    outT = np.stack(raw_outs)  # [BS, NH, FOUT+1, N]
    num = outT[:, :, 0:FOUT, :]
    den = outT[:, :, FOUT : FOUT + 1, :]
    return np.ascontiguousarray((num / den).transpose(0, 1, 3, 2)).astype(np.float32)


_NC_CACHE = {}


def kernel(h, adj, w, a_src, a_dst):
    if "nc" not in _NC_CACHE:
        _NC_CACHE["nc"] = build_program(num_devices=BS)
    nc = _NC_CACHE["nc"]
    in_maps = make_in_maps(h, adj, w, a_src, a_dst)
    res = run_bass_kernel_spmd(nc, in_maps, core_ids=list(range(BS)))
    return postprocess([r["outT"] for r in res.results])
